# revision 6
# baseline (speedup 1.0000x reference)
"""DeepSeek MoE layer on 8 Trainium2 NeuronCores (Bass/Tile).

Sharding: expert parallelism. Core c owns routed experts 4c..4c+3 and a
256-wide slice of the shared experts' intermediate dim. The gate (routing)
is replicated on every core. Each core produces a partial output in
transposed layout [H, T]; the host sums the 8 partials and transposes.

Matmuls run in float32r (fp32 bits, full-rate PE mode). All compute is
dense over tokens: routed expert outputs are weighted by the dense [T, E]
combine matrix computed on-device (grouped top-k routing, exactly as the
reference).
"""

import sys

sys.path.insert(0, "/opt/trn_rl_repo")

import numpy as np

import concourse.bass as bass  # noqa: F401  (import establishes package state)
import concourse.mybir as mybir
import concourse.tile as tile
from concourse import bacc
from concourse.bass_utils import run_bass_kernel_spmd
from concourse.masks import make_identity

F32 = mybir.dt.float32
F32R = mybir.dt.float32r
AF = mybir.ActivationFunctionType
ALU = mybir.AluOpType

# Problem constants (hardcoded per contract).
T = 512       # tokens
H = 2048      # hidden
I = 1024      # moe intermediate
I2 = 2 * I    # gate+up cols per expert
E = 32        # routed experts
K = 8         # experts per token
NG = 8        # routing groups
TG = 4        # top-k groups
SCALE = 2.5   # routed scaling factor
NCORES = 8
EL = E // NCORES          # local experts per core = 4
SI = 256                  # shared-intermediate slice per core (2*1024/8)
P = 128
HK = H // P               # 16 k-tiles over hidden
TM = T // P               # 4 token tiles
IK = I // P               # 8 k-tiles over intermediate
NEG1 = -1.0e30
NEG2 = -2.0e30


def _r(ap):
    """View an fp32 AP as float32r for full-rate PE matmul."""
    return ap.bitcast(F32R)


def _build_body(tc, d, pools):
    nc = tc.nc
    sb, work, stream, psA, psB = pools

    ident = sb.tile([P, P], F32, name="ident")
    make_identity(nc, ident)

    # ---- x [T, H] -> xT [128, HK, T] (hidden on partitions) ----
    xT = sb.tile([P, HK, T], F32R, name="xT")
    for m in range(TM):
        xst = work.tile([P, H], F32, tag="xstage")
        nc.sync.dma_start(xst[:], d["x"][m * P:(m + 1) * P, :])
        for k in range(HK):
            pt = psB.tile([P, P], F32, tag="small")
            nc.tensor.transpose(pt[:], xst[:, k * P:(k + 1) * P], ident[:])
            nc.vector.tensor_copy(xT[:, k, m * P:(m + 1) * P], pt[:])

    # ---- gate_w [E, H] -> gwT [128, HK, E] ----
    gwT = sb.tile([P, HK, E], F32R, name="gwT")
    gwS = sb.tile([E, H], F32, name="gwS")
    nc.sync.dma_start(gwS[:], d["gw"][:])
    for k in range(HK):
        pt = psB.tile([P, E], F32, tag="small")
        nc.tensor.transpose(pt[:], gwS[:, k * P:(k + 1) * P], ident[:E, :E])
        nc.vector.tensor_copy(gwT[:, k, :], pt[:])

    # bias broadcast [128, E] comes precomputed from host
    gbb = sb.tile([P, E], F32, name="gbb")
    nc.sync.dma_start(gbb[:], d["gbb"][:])

    neg = sb.tile([P, E], F32, name="neg")
    nc.vector.memset(neg[:], NEG1)

    # ---- routing -> cwT [E, T] (dense combine weights, transposed) ----
    cwT = sb.tile([E, T], F32R, name="cwT")
    for m in range(TM):
        plg = psB.tile([P, E], F32, tag="small")
        for k in range(HK):
            nc.tensor.matmul(
                plg[:],
                xT[:, k, m * P:(m + 1) * P],
                gwT[:, k, :],
                start=(k == 0),
                stop=(k == HK - 1),
            )
        s_t = work.tile([P, E], F32, tag="s_t")
        nc.scalar.activation(s_t[:], plg[:], AF.Sigmoid)
        sc = work.tile([P, E], F32, tag="sc")
        nc.vector.tensor_add(sc[:], s_t[:], gbb[:])

        # group score: sum of top-2 within each group of 4
        sc3 = sc[:].rearrange("p (g f) -> p g f", f=4)
        ga = work.tile([P, NG], F32, tag="ga")
        gb_ = work.tile([P, NG], F32, tag="gb_")
        gc = work.tile([P, NG], F32, tag="gc")
        gd = work.tile([P, NG], F32, tag="gd")
        # pair maxes/mins
        nc.vector.tensor_tensor(ga[:], sc3[:, :, 0], sc3[:, :, 1], ALU.max)
        nc.vector.tensor_tensor(gb_[:], sc3[:, :, 0], sc3[:, :, 1], ALU.min)
        nc.vector.tensor_tensor(gc[:], sc3[:, :, 2], sc3[:, :, 3], ALU.max)
        nc.vector.tensor_tensor(gd[:], sc3[:, :, 2], sc3[:, :, 3], ALU.min)
        hi = work.tile([P, NG], F32, tag="hi")
        lo = work.tile([P, NG], F32, tag="lo")
        mid = work.tile([P, NG], F32, tag="mid")
        nc.vector.tensor_tensor(hi[:], ga[:], gc[:], ALU.max)
        nc.vector.tensor_tensor(lo[:], ga[:], gc[:], ALU.min)
        nc.vector.tensor_tensor(mid[:], gb_[:], gd[:], ALU.max)
        gsc = work.tile([P, NG], F32, tag="gsc")
        nc.vector.tensor_tensor(gsc[:], lo[:], mid[:], ALU.max)
        nc.vector.tensor_add(gsc[:], gsc[:], hi[:])

        # top-TG groups -> 0/1 mask over groups
        gm8 = work.tile([P, 8], F32, tag="gm8")
        nc.vector.max(gm8[:], gsc[:])
        nc.vector.memset(gm8[:, TG:], NEG1)
        gz = work.tile([P, NG], F32, tag="gz")
        nc.vector.match_replace(out=gz[:], in_to_replace=gm8[:], in_values=gsc[:], imm_value=NEG1)
        gmask = work.tile([P, NG], mybir.dt.uint32, tag="gmask")
        nc.vector.tensor_scalar(gmask[:], gz[:], -5.0e29, None, op0=ALU.is_le)

        # expand to experts, mask scores
        emask = work.tile([P, E], mybir.dt.uint32, tag="emask")
        em3 = emask[:].rearrange("p (g f) -> p g f", f=4)
        nc.vector.tensor_copy(em3[:], gmask[:, :, None].to_broadcast([P, NG, 4]))
        msk = work.tile([P, E], F32, tag="msk")
        nc.vector.select(out=msk[:], mask=emask[:], on_true=sc[:], on_false=neg[:])

        # top-K experts -> 0/1 selection mask
        t8 = work.tile([P, 8], F32, tag="t8")
        nc.vector.max(t8[:], msk[:])
        mz = work.tile([P, E], F32, tag="mz")
        nc.vector.match_replace(out=mz[:], in_to_replace=t8[:], in_values=msk[:], imm_value=NEG2)
        sel = work.tile([P, E], F32, tag="selm")
        nc.vector.tensor_scalar(sel[:], mz[:], -1.5e30, None, op0=ALU.is_le)

        # weights: s * sel, renormalized, * SCALE
        wr = work.tile([P, E], F32, tag="wr")
        nc.vector.tensor_mul(wr[:], s_t[:], sel[:])
        ws = work.tile([P, 1], F32, tag="ws")
        nc.vector.reduce_sum(ws[:], wr[:], axis=mybir.AxisListType.X)
        rec = work.tile([P, 1], F32, tag="rec")
        nc.vector.reciprocal(rec[:], ws[:])
        coef = work.tile([P, 1], F32, tag="coef")
        nc.vector.tensor_scalar_mul(coef[:], rec[:], SCALE)
        cw_t = work.tile([P, E], F32, tag="cw_t")
        nc.vector.tensor_scalar_mul(cw_t[:], wr[:], coef[:])

        ptc = psB.tile([E, P], F32, tag="small")
        nc.tensor.transpose(ptc[:], cw_t[:], ident[:])
        nc.vector.tensor_copy(cwT[:, m * P:(m + 1) * P], ptc[:])

    # ---- broadcast local experts' combine rows: cwb [128, EL, T] ----
    bselS = sb.tile([E, EL * P], F32R, name="bselS")
    nc.sync.dma_start(bselS[:], d["bsel"][:])
    cwb = sb.tile([P, EL, T], F32, name="cwb")
    for j in range(EL):
        pb = psA.tile([P, T], F32, tag="mm")
        nc.tensor.matmul(pb[:], bselS[:, j * P:(j + 1) * P], cwT[:], start=True, stop=True)
        nc.vector.tensor_copy(cwb[:, j, :], pb[:])

    outT = sb.tile([P, HK, T], F32, name="outT")

    # ---- routed experts ----
    for j in range(EL):
        sg = work.tile([P, IK, T], F32, tag="sg")
        actw = work.tile([P, IK, T], F32R, tag="actw")
        # gate_up: out cols 0..I are gate, I..2I are up; quarters of 4 i-tiles
        for q in range(4):
            pps = [psA.tile([P, T], F32, tag="mm", name=f"pps{i}") for i in range(4)]
            for k in range(HK):
                wst = stream.tile([P, 512], F32R, tag="wstream")
                nc.sync.dma_start(
                    wst[:], d["wgu"][j, k * P:(k + 1) * P, q * 512:(q + 1) * 512]
                )
                for i in range(4):
                    nc.tensor.matmul(
                        pps[i][:],
                        wst[:, i * P:(i + 1) * P],
                        xT[:, k, :],
                        start=(k == 0),
                        stop=(k == HK - 1),
                    )
            if q < 2:
                for i in range(4):
                    it = 4 * q + i
                    sgm = work.tile([P, T], F32, tag="sgm")
                    nc.scalar.activation(sgm[:], pps[i][:], AF.Sigmoid)
                    nc.vector.tensor_mul(sg[:, it, :], sgm[:], pps[i][:])
            else:
                for i in range(4):
                    it = 4 * (q - 2) + i
                    atmp = work.tile([P, T], F32, tag="atmp")
                    nc.vector.tensor_mul(atmp[:], sg[:, it, :], pps[i][:])
                    nc.vector.tensor_mul(actw[:, it, :], atmp[:], cwb[:, j, :])
        # down: accumulate into outT (copy on first expert)
        for hq in range(4):
            ppd = [psA.tile([P, T], F32, tag="mm", name=f"ppd{i}") for i in range(4)]
            for i2 in range(IK):
                wds = stream.tile([P, 512], F32R, tag="wstream")
                nc.sync.dma_start(
                    wds[:], d["wd"][j, i2 * P:(i2 + 1) * P, hq * 512:(hq + 1) * 512]
                )
                for h in range(4):
                    nc.tensor.matmul(
                        ppd[h][:],
                        wds[:, h * P:(h + 1) * P],
                        actw[:, i2, :],
                        start=(i2 == 0),
                        stop=(i2 == IK - 1),
                    )
            for h in range(4):
                ht = 4 * hq + h
                if j == 0:
                    nc.vector.tensor_copy(outT[:, ht, :], ppd[h][:])
                else:
                    nc.vector.tensor_add(outT[:, ht, :], outT[:, ht, :], ppd[h][:])

    # ---- shared experts (this core's 256-wide intermediate slice) ----
    # swgu [H, 512] packed [gate(256) | up(256)]
    pss = [psA.tile([P, T], F32, tag="mm", name=f"pss{i}") for i in range(4)]
    for k in range(HK):
        sws = stream.tile([P, 512], F32R, tag="wstream")
        nc.sync.dma_start(sws[:], d["swgu"][k * P:(k + 1) * P, :])
        for i in range(4):
            nc.tensor.matmul(
                pss[i][:],
                sws[:, i * P:(i + 1) * P],
                xT[:, k, :],
                start=(k == 0),
                stop=(k == HK - 1),
            )
    acts = work.tile([P, 2, T], F32R, tag="acts")
    for t in range(2):
        sst = work.tile([P, T], F32, tag="sst")
        nc.scalar.activation(sst[:], pss[t][:], AF.Sigmoid)
        nc.vector.tensor_mul(sst[:], sst[:], pss[t][:])
        nc.vector.tensor_mul(acts[:, t, :], sst[:], pss[2 + t][:])
    # shared down: swd [256, H]
    for hq in range(4):
        ppd = [psA.tile([P, T], F32, tag="mm", name=f"ppd{i}") for i in range(4)]
        for i2 in range(2):
            wds = stream.tile([P, 512], F32R, tag="wstream")
            nc.sync.dma_start(
                wds[:], d["swd"][i2 * P:(i2 + 1) * P, hq * 512:(hq + 1) * 512]
            )
            for h in range(4):
                nc.tensor.matmul(
                    ppd[h][:],
                    wds[:, h * P:(h + 1) * P],
                    acts[:, i2, :],
                    start=(i2 == 0),
                    stop=(i2 == 1),
                )
        for h in range(4):
            ht = 4 * hq + h
            nc.vector.tensor_add(outT[:, ht, :], outT[:, ht, :], ppd[h][:])

    nc.sync.dma_start(d["outT"].rearrange("(ho p) t -> p ho t", p=P), outT[:])


def build_nc(repeat=1):
    """Build and compile the per-core Bass program. repeat>1 wraps the body
    in a hardware loop (for wall-clock timing of the kernel proper)."""
    nc = bacc.Bacc("TRN2", target_bir_lowering=False, debug=False, num_devices=NCORES)
    d = {
        "x": nc.dram_tensor("x", [T, H], F32, kind="ExternalInput").ap(),
        "gw": nc.dram_tensor("gw", [E, H], F32, kind="ExternalInput").ap(),
        "gbb": nc.dram_tensor("gbb", [P, E], F32, kind="ExternalInput").ap(),
        "bsel": nc.dram_tensor("bsel", [E, EL * P], F32R, kind="ExternalInput").ap(),
        "wgu": nc.dram_tensor("wgu", [EL, H, I2], F32R, kind="ExternalInput").ap(),
        "wd": nc.dram_tensor("wd", [EL, I, H], F32R, kind="ExternalInput").ap(),
        "swgu": nc.dram_tensor("swgu", [H, 2 * SI], F32R, kind="ExternalInput").ap(),
        "swd": nc.dram_tensor("swd", [SI, H], F32R, kind="ExternalInput").ap(),
        "outT": nc.dram_tensor("outT", [H, T], F32, kind="ExternalOutput").ap(),
    }
    with tile.TileContext(nc) as tc:
        with (
            tc.tile_pool(name="sb", bufs=1) as sb,
            tc.tile_pool(name="work", bufs=2) as work,
            tc.tile_pool(name="stream", bufs=4) as stream,
            tc.tile_pool(name="psA", bufs=6, space="PSUM") as psA,
            tc.tile_pool(name="psB", bufs=2, space="PSUM") as psB,
        ):
            pools = (sb, work, stream, psA, psB)
            if repeat == 1:
                _build_body(tc, d, pools)
            else:
                with tc.For_i(0, repeat, 1):
                    _build_body(tc, d, pools)
    nc.compile()
    return nc


def shard_inputs(hidden_states, gate_w, gate_bias, w_gate_up, w_down,
                 shared_w_gate_up, shared_w_down):
    """Build the per-core input maps."""
    x = np.ascontiguousarray(hidden_states, dtype=np.float32)
    gw = np.ascontiguousarray(gate_w, dtype=np.float32)
    gbb = np.tile(np.asarray(gate_bias, dtype=np.float32)[None, :], (P, 1))
    gbb = np.ascontiguousarray(gbb)
    in_maps = []
    for c in range(NCORES):
        bsel = np.zeros((E, EL * P), dtype=np.float32)
        for j in range(EL):
            bsel[EL * c + j, j * P:(j + 1) * P] = 1.0
        swgu = np.concatenate(
            [
                shared_w_gate_up[:, c * SI:(c + 1) * SI],
                shared_w_gate_up[:, 2 * I + c * SI: 2 * I + (c + 1) * SI],
            ],
            axis=1,
        )
        in_maps.append({
            "x": x,
            "gw": gw,
            "gbb": gbb,
            "bsel": bsel,
            "wgu": np.ascontiguousarray(w_gate_up[EL * c:EL * (c + 1)], dtype=np.float32),
            "wd": np.ascontiguousarray(w_down[EL * c:EL * (c + 1)], dtype=np.float32),
            "swgu": np.ascontiguousarray(swgu, dtype=np.float32),
            "swd": np.ascontiguousarray(shared_w_down[c * SI:(c + 1) * SI, :], dtype=np.float32),
        })
    return in_maps


_NC_CACHE = {}


def kernel(hidden_states, gate_w, gate_bias, w_gate_up, w_down,
           shared_w_gate_up, shared_w_down):
    if "nc" not in _NC_CACHE:
        _NC_CACHE["nc"] = build_nc(repeat=1)
    nc = _NC_CACHE["nc"]
    in_maps = shard_inputs(hidden_states, gate_w, gate_bias, w_gate_up, w_down,
                           shared_w_gate_up, shared_w_down)
    res = run_bass_kernel_spmd(nc, in_maps, list(range(NCORES)))
    acc = np.zeros((H, T), dtype=np.float32)
    for c in range(NCORES):
        acc += res.results[c]["outT"]
    return np.ascontiguousarray(acc.T)
